# revision 20
# baseline (speedup 1.0000x reference)
"""Trainium2 Bass kernel for nn_MultiHeadAttention (B=4, S=2048, D=1024, H=16).

Sharding: 8 cores = 4 batches x 2 head-halves (8 heads each). No collectives:
each core computes Q/K/V projections for its (batch, 8-head) slice, attention
scores in transposed [k, q] layout (softmax sums via ones-matmuls, no max
subtraction -- scores are O(6) so exp is safe in fp32), attention weights
written to DRAM as [h, k, q] (host transposes views on assembly), AV and the
output projection per-core with the softmax normalization applied on-chip.
Host sums the two half-head partial outputs per batch (the "all-reduce").

Matmul inputs are bf16 (full TensorE rate; fp32 matmul is 4x slower);
accumulation is fp32 in PSUM. Softmax/normalization arithmetic is fp32.
"""

import os
import numpy as np
import ml_dtypes

import concourse.bass as bass
import concourse.bacc as bacc
import concourse.tile as tile
from concourse import mybir
from concourse.bass_utils import run_bass_kernel_spmd

BF16 = mybir.dt.bfloat16
F32 = mybir.dt.float32
NPBF16 = ml_dtypes.bfloat16

# Problem constants (full size; build_nc is parameterized for sim testing)
B = 4
S_FULL = 2048
D_MODEL = 1024
NUM_HEADS = 16
DEPTH = 64
N_CORES = 8
HPC = 8            # heads per core
FPC = HPC * DEPTH  # features per core = 512
PAIRS = HPC // 2   # head pairs per core = 4
SCALE = 1.0 / np.sqrt(np.float32(DEPTH))  # folded into exp's scale operand

# module-level stash so test.py can inspect the raw run (exec time etc)
LAST_RESULT = None


def build_nc(S=S_FULL, D=D_MODEL, hpc=HPC, debug=False):
    """Build the single-core SPMD Bass program (identical on all 8 cores)."""
    ICH = D // 128          # input-feature chunks (contraction) = 8
    fpc = hpc * DEPTH       # features per core
    pairs = hpc // 2
    TB512 = S // 512        # 512-token blocks = 4
    TB128 = S // 128        # 128-token blocks = 16
    QB = S // 512           # query blocks = 4
    KB = S // 128           # key blocks = 16
    KBG = KB // 2           # key block groups (2 kb per exp) = 8

    nc = bacc.Bacc("TRN2", target_bir_lowering=False, debug=debug,
                   num_devices=N_CORES)

    # ---- DRAM I/O ----
    xqt = nc.dram_tensor("xqt", [D, S], BF16, kind="ExternalInput").ap()
    xkt = nc.dram_tensor("xkt", [D, S], BF16, kind="ExternalInput").ap()
    xvt = nc.dram_tensor("xvt", [D, S], BF16, kind="ExternalInput").ap()
    wqt = nc.dram_tensor("wqt", [D, fpc], BF16, kind="ExternalInput").ap()
    wkt = nc.dram_tensor("wkt", [D, fpc], BF16, kind="ExternalInput").ap()
    wvt = nc.dram_tensor("wvt", [D, fpc], BF16, kind="ExternalInput").ap()
    wot = nc.dram_tensor("wot", [fpc, D], BF16, kind="ExternalInput").ap()
    bq = nc.dram_tensor("bq", [fpc], BF16, kind="ExternalInput").ap()
    bk = nc.dram_tensor("bk", [fpc], BF16, kind="ExternalInput").ap()
    bv = nc.dram_tensor("bv", [fpc], BF16, kind="ExternalInput").ap()
    bo = nc.dram_tensor("bo", [D], BF16, kind="ExternalInput").ap()

    wt_out = nc.dram_tensor("wt_out", [hpc, S, S], F32, kind="ExternalOutput").ap()
    y_out = nc.dram_tensor("y_out", [S, D], F32, kind="ExternalOutput").ap()

    with tile.TileContext(nc) as tc:
        with (
            tc.tile_pool(name="persist", bufs=1) as persist,
            tc.tile_pool(name="small", bufs=2) as small,
        ):
            # persistent SBUF tensors
            qt_sb = persist.tile([128, pairs, S], BF16)   # Q.T  (feat-major)
            kt_sb = persist.tile([128, pairs, S], BF16)   # K.T  (feat-major)
            v_sb = persist.tile([128, TB128, hpc, DEPTH], BF16)  # V (token-major)
            wq_sb = persist.tile([128, ICH, fpc], BF16)
            wk_sb = persist.tile([128, ICH, fpc], BF16)
            wv_sb = persist.tile([128, ICH, fpc], BF16)
            wot_sb = persist.tile([128, pairs, D], BF16)
            bq_sb = persist.tile([1, fpc], BF16)
            bk_sb = persist.tile([1, fpc], BF16)
            bv_sb = persist.tile([1, fpc], BF16)
            bo_sb = persist.tile([1, D], BF16)
            ones_c = persist.tile([128, 1], BF16)   # sums-matmul stationary
            ones_r = persist.tile([33, 128], BF16)  # bias/broadcast stationary
            zrow = persist.tile([1, 512], BF16)     # zero row for PSUM open/close
            onesrow = persist.tile([1, 512], BF16)  # ones row for bias matmuls

            nc.vector.memset(ones_c, 1.0)
            nc.vector.memset(ones_r, 1.0)
            nc.vector.memset(zrow, 0.0)
            nc.vector.memset(onesrow, 1.0)

            # weight/bias loads
            nc.gpsimd.dma_start(out=wq_sb, in_=wqt.rearrange("(i p) f -> p i f", p=128))
            nc.gpsimd.dma_start(out=wk_sb, in_=wkt.rearrange("(i p) f -> p i f", p=128))
            nc.gpsimd.dma_start(out=wv_sb, in_=wvt.rearrange("(i p) f -> p i f", p=128))
            nc.gpsimd.dma_start(out=wot_sb, in_=wot.rearrange("(c p) o -> p c o", p=128))
            nc.gpsimd.dma_start(out=bq_sb, in_=bq[None, :])
            nc.gpsimd.dma_start(out=bk_sb, in_=bk[None, :])
            nc.gpsimd.dma_start(out=bv_sb, in_=bv[None, :])
            nc.gpsimd.dma_start(out=bo_sb, in_=bo[None, :])

            # ---------------- Phase A: projections ----------------
            with (
                tc.tile_pool(name="xt", bufs=2) as xt_pool,
                tc.tile_pool(name="psA", bufs=4, space="PSUM") as psA,
            ):
                for x_dram, w_sb, b_sb, kind in (
                    (xqt, wq_sb, bq_sb, "q"),
                    (xkt, wk_sb, bk_sb, "k"),
                    (xvt, wv_sb, None, "v"),
                ):
                    xt = xt_pool.tile([128, ICH, S], BF16, tag="xt")
                    nc.gpsimd.dma_start(
                        out=xt, in_=x_dram.rearrange("(i p) s -> p i s", p=128))
                    if kind in ("q", "k"):
                        dst = qt_sb if kind == "q" else kt_sb
                        for pr in range(pairs):
                            for tb in range(TB512):
                                ps = psA.tile([128, 512], F32, tag="projps")
                                for i in range(ICH):
                                    nc.tensor.matmul(
                                        ps,
                                        lhsT=w_sb[:, i, pr * 128:(pr + 1) * 128],
                                        rhs=xt[:, i, tb * 512:(tb + 1) * 512],
                                        start=(i == 0), stop=False)
                                # bias: out[f, t] += b[f] * 1
                                nc.tensor.matmul(
                                    ps, lhsT=b_sb[:, pr * 128:(pr + 1) * 128],
                                    rhs=onesrow, start=False, stop=True)
                                nc.vector.tensor_copy(
                                    out=dst[:, pr, tb * 512:(tb + 1) * 512], in_=ps)
                    else:
                        for tb in range(TB128):
                            ps = psA.tile([128, fpc], F32, tag="vps")
                            for i in range(ICH):
                                nc.tensor.matmul(
                                    ps,
                                    lhsT=xt[:, i, tb * 128:(tb + 1) * 128],
                                    rhs=w_sb[:, i, :],
                                    start=(i == 0), stop=False)
                            nc.tensor.matmul(
                                ps, lhsT=ones_r[0:1, :], rhs=bv_sb,
                                start=False, stop=True)
                            nc.vector.tensor_copy(
                                out=v_sb[:, tb],
                                in_=ps.rearrange("p (h d) -> p h d", d=DEPTH))

            # ---------------- Phase B: attention + output proj ----------------
            with (
                tc.tile_pool(name="et", bufs=2) as etp,
                tc.tile_pool(name="ast", bufs=3) as astp,
                tc.tile_pool(name="avtn", bufs=2) as avtnp,
                tc.tile_pool(name="ysb", bufs=2) as ypool,
                tc.tile_pool(name="psB", bufs=1, space="PSUM") as psB,
            ):
                for qb in range(QB):
                    avtn = avtnp.tile([128, pairs, 512], BF16, tag="avtn")
                    for pr in range(pairs):
                        et0 = etp.tile([128, KB, 512], BF16, tag="et0")
                        et1 = etp.tile([128, KB, 512], BF16, tag="et1")
                        ets = (et0, et1)
                        avt = psB.tile([128, 512], F32, tag="avt")
                        sums = psB.tile([33, 512], F32, tag="sums")
                        # open accumulation over the full bank height (the two
                        # heads write disjoint partition halves; hw/sim track
                        # psum groups at 2KB-bank granularity)
                        nc.tensor.matmul(avt, lhsT=ones_r[0:1, :], rhs=zrow,
                                         start=True, stop=False)
                        nc.tensor.matmul(sums, lhsT=ones_r[0:1, 0:33], rhs=zrow,
                                         start=True, stop=False)
                        for kbg in range(KBG):
                            for h01 in (0, 1):
                                et = ets[h01]
                                sc = psB.tile([128, 2, 512], F32, tag=f"sc{h01}")
                                for j in (0, 1):
                                    kb = kbg * 2 + j
                                    nc.tensor.matmul(
                                        sc[:, j, :],
                                        lhsT=kt_sb[h01 * 64:(h01 + 1) * 64, pr,
                                                   kb * 128:(kb + 1) * 128],
                                        rhs=qt_sb[h01 * 64:(h01 + 1) * 64, pr,
                                                  qb * 512:(qb + 1) * 512],
                                        start=True, stop=True,
                                        tile_position=(h01 * 64, 0))
                                nc.scalar.activation(
                                    out=et[:, kbg * 2:kbg * 2 + 2, :],
                                    in_=sc,
                                    func=mybir.ActivationFunctionType.Exp,
                                    scale=float(SCALE))
                                for j in (0, 1):
                                    kb = kbg * 2 + j
                                    h = pr * 2 + h01
                                    nc.tensor.matmul(
                                        avt[h01 * 64:(h01 + 1) * 64, :],
                                        lhsT=v_sb[:, kb, h, :],
                                        rhs=et[:, kb, :],
                                        start=False, stop=False,
                                        tile_position=(0, h01 * 64))
                                    nc.tensor.matmul(
                                        sums[h01 * 32:h01 * 32 + 1, :],
                                        lhsT=ones_c,
                                        rhs=et[:, kb, :],
                                        start=False, stop=False,
                                        tile_position=(0, h01 * 32))

                        # close the accumulation groups (numeric no-ops)
                        nc.tensor.matmul(avt, lhsT=ones_r[0:1, :], rhs=zrow,
                                         start=False, stop=True)
                        nc.tensor.matmul(sums, lhsT=ones_r[0:1, 0:33], rhs=zrow,
                                         start=False, stop=True)

                        # epilogue: 1/sums, broadcast, normalize, write weights
                        recip = small.tile([33, 512], F32, tag="recip")
                        recip_bf = small.tile([33, 512], BF16, tag="recipbf")
                        for h01 in (0, 1):
                            r = slice(h01 * 32, h01 * 32 + 1)
                            if os.environ.get("MHA_SAFE_RECIP"):
                                nc.vector.reciprocal(out=recip[r, :], in_=sums[r, :])
                            else:
                                nc.vector.reciprocal_approx_fast(
                                    out=recip[r, :], in_=sums[r, :])
                            nc.vector.tensor_copy(out=recip_bf[r, :], in_=recip[r, :])
                        for h01 in (0, 1):
                            h = pr * 2 + h01
                            et = ets[h01]
                            bc_ps = psB.tile([128, 512], F32, tag="bc")
                            nc.tensor.matmul(
                                bc_ps, lhsT=ones_r[h01 * 32:h01 * 32 + 1, :],
                                rhs=recip_bf[h01 * 32:h01 * 32 + 1, :],
                                start=True, stop=True)
                            bc = small.tile([128, 512], BF16, tag=f"bc{h01}")
                            nc.vector.tensor_copy(out=bc, in_=bc_ps)
                            hs = slice(h01 * 64, (h01 + 1) * 64)
                            nc.vector.tensor_mul(
                                out=avtn[hs, pr, :], in0=avt[hs, :], in1=bc[hs, :])
                            # weights: A.T chunk = E.T * (1/s) broadcast, fp32 out
                            bcap = bc[:]
                            for cg in range(KB // 4):
                                ast = astp.tile([128, 4, 512], F32, tag="ast")
                                bc_b = bass.AP(
                                    tensor=bcap.tensor, offset=bcap.offset,
                                    ap=[bcap.ap[0], [0, 4], bcap.ap[1]])
                                nc.vector.tensor_mul(
                                    out=ast, in0=et[:, cg * 4:(cg + 1) * 4, :],
                                    in1=bc_b)
                                nc.sync.dma_start(
                                    out=wt_out[h, cg * 512:(cg + 1) * 512,
                                               qb * 512:(qb + 1) * 512]
                                    .rearrange("(c p) q -> p c q", p=128),
                                    in_=ast)

                    # output projection for this query block
                    for q2 in range(4):
                        ysb = ypool.tile([128, 1024], F32, tag="ysb")
                        for oh in (0, 1):
                            ys = psB.tile([128, 512], F32, tag="yps")
                            for p4 in range(pairs):
                                nc.tensor.matmul(
                                    ys,
                                    lhsT=avtn[:, p4, q2 * 128:(q2 + 1) * 128],
                                    rhs=wot_sb[:, p4, oh * 512:(oh + 1) * 512],
                                    start=(p4 == 0), stop=False)
                            nc.tensor.matmul(
                                ys, lhsT=ones_r[0:1, :],
                                rhs=bo_sb[:, oh * 512:(oh + 1) * 512],
                                start=False, stop=True)
                            nc.vector.tensor_copy(
                                out=ysb[:, oh * 512:(oh + 1) * 512], in_=ys)
                        q0 = qb * 512 + q2 * 128
                        nc.sync.dma_start(out=y_out[q0:q0 + 128, :], in_=ysb)

    nc.compile()
    return nc


def make_in_maps(query, key_, value, Wq, bq, Wk, bk, Wv, bv, Wo, bo,
                 S=S_FULL, D=D_MODEL, hpc=HPC, n_cores=N_CORES):
    """Host-side shard/cast/transpose into per-core input maps."""
    fpc = hpc * DEPTH
    q16 = np.ascontiguousarray(np.asarray(query).astype(NPBF16).transpose(0, 2, 1))
    k16 = np.ascontiguousarray(np.asarray(key_).astype(NPBF16).transpose(0, 2, 1))
    v16 = np.ascontiguousarray(np.asarray(value).astype(NPBF16).transpose(0, 2, 1))
    WqT = np.ascontiguousarray(np.asarray(Wq).T.astype(NPBF16))  # [D, D]
    WkT = np.ascontiguousarray(np.asarray(Wk).T.astype(NPBF16))
    WvT = np.ascontiguousarray(np.asarray(Wv).T.astype(NPBF16))
    WoT = np.ascontiguousarray(np.asarray(Wo).T.astype(NPBF16))  # [D, D] (in, out)
    bq = np.asarray(bq).astype(NPBF16)
    bk = np.asarray(bk).astype(NPBF16)
    bv16 = np.asarray(bv).astype(NPBF16)
    bo16 = np.asarray(bo).astype(NPBF16)
    zeros_bo = np.zeros_like(bo16)

    in_maps = []
    for c in range(n_cores):
        b, g = c // 2, c % 2
        fs = slice(g * fpc, (g + 1) * fpc)
        in_maps.append({
            "xqt": q16[b], "xkt": k16[b], "xvt": v16[b],
            "wqt": np.ascontiguousarray(WqT[:, fs]),
            "wkt": np.ascontiguousarray(WkT[:, fs]),
            "wvt": np.ascontiguousarray(WvT[:, fs]),
            "wot": np.ascontiguousarray(WoT[fs, :]),
            "bq": np.ascontiguousarray(bq[fs]),
            "bk": np.ascontiguousarray(bk[fs]),
            "bv": np.ascontiguousarray(bv16[fs]),
            "bo": bo16 if g == 0 else zeros_bo,
        })
    return in_maps


def assemble(results, B_=B, S=S_FULL, D=D_MODEL, hpc=HPC, n_heads=NUM_HEADS):
    """Gather per-core outputs into (out, attention_weights)."""
    out = np.empty((B_, S, D), np.float32)
    attw = np.empty((B_, n_heads, S, S), np.float32)
    for b in range(B_):
        out[b] = results[2 * b]["y_out"]
        out[b] += results[2 * b + 1]["y_out"]
        for g in range(2):
            wt = results[2 * b + g]["wt_out"]  # [hpc, k, q]
            attw[b, g * hpc:(g + 1) * hpc] = wt.transpose(0, 2, 1)
    return out, attw


def run(inputs, trace=False, trace_kwargs=None):
    global LAST_RESULT
    nc = build_nc()
    in_maps = make_in_maps(**inputs)
    res = run_bass_kernel_spmd(
        nc, in_maps, core_ids=list(range(N_CORES)), trace=trace,
        **(trace_kwargs or {}))
    LAST_RESULT = res
    return assemble(res.results)


def kernel(query, key_, value, Wq, bq, Wk, bk, Wv, bv, Wo, bo):
    return run(dict(query=query, key_=key_, value=value, Wq=Wq, bq=bq,
                    Wk=Wk, bk=bk, Wv=Wv, bv=bv, Wo=Wo, bo=bo))
